# revision 3
# baseline (speedup 1.0000x reference)
"""GAT additive-attention kernel (nn_GAT) for 8 Trainium2 NeuronCores.

reference:
    k = x @ w_k; q = x @ w_q                      # [bz, N, 1]
    s[b,i,j]   = leaky_relu(k[b,i] + q[b,j], 0.2)
    attn       = softmax(s, axis=j)
    out        = (attn @ x).transpose(0, 2, 1)    # [bz, F, N]

Key identity: with sigma[i,j] = sign(k_i + q_j),
    exp(lrelu(s)) = exp(k_i)exp(q_j)       if s > 0
                  = exp(.2 k_i)exp(.2 q_j) if s <= 0
so with U = exp(q_j - qm)*[x_j|1], V = exp(.2(q_j - qm))*[x_j|1]:
    out_i = num_i / den_i,  [num|den]_i = (M@U)_i + e_i * (colsum(V) - (M@V))_i
where M = (sigma+1)/2 and e_i = exp(-.8 (k_i + qm)).  Everything reduces
to ONE N x N masked matmul  T = sigma-ish @ [U|V]  (fp16, rescaled by the
global qm = max q so fp16 never overflows) plus O(N*F) pre/post work.

v2 schedule (vs v1): the host pre-broadcasts wq/wk so the kernel needs no
weight-broadcast matmuls; k_bcast comes from ONE matmul stage (wk replicated
as the stationary operand against x^T); q is built chunk-by-chunk with
accumulating scalar_tensor_tensor as the x DMA lands; gamma/delta column
sums run right after W is built (not at stream end) so the whole G
correction is ready mid-stream; the G add is folded into the PSUM
evacuation (tensor_tensor add, PSUM src is 1x anyway); the post phase uses
wide 3D ops instead of per-chunk ops; masks are produced on three engines
(ACT / DVE / GPSIMD) so production always stays ahead of the matmul stream.

All 16 row-chunk accumulators live in PSUM simultaneously by packing
2-3 accumulation groups per bank: start=True (which clears the WHOLE
bank's has_written bits) is only used by the first group touching each
bank; later groups open with start=False, which overwrites where the
bits are clear and accumulates afterwards.

Sharding: core c handles batch b = c//2, row-half h = c%2 (2048 rows),
fully data-parallel (no collectives).
"""

import sys
import numpy as np

for _p in ("/opt/trn_rl_repo",):
    if _p not in sys.path:
        sys.path.insert(0, _p)

N = 4096
F = 64
BZ = 4
HALF = 2048
NCH = 32          # j-chunks of 128 (full N)
NIC = 16          # i-chunks of 128 (this core's half)
NEG_SLOPE = 0.2
SLOT_OFF = 176    # fp32 elems between accumulation groups within a bank

ACTSET = frozenset(c for c in range(NCH) if c % 4 == 0)   # sign masks, slot A
GPSET = frozenset(c for c in range(NCH) if c % 4 == 2)    # gpsimd masks, slot B

_CACHE = {}


def _body(nc, tc):
    import dataclasses
    import concourse.mybir as mybir
    from concourse import bass_isa

    f32 = mybir.dt.float32
    f16 = mybir.dt.float16
    bf16 = mybir.dt.bfloat16
    Alu = mybir.AluOpType
    Act = mybir.ActivationFunctionType
    Ax = mybir.AxisListType

    # host-prepacked inputs (see make_in_maps)
    xfp_d = nc.dram_tensor("xfp", [128, NCH * F], f32, kind="ExternalInput").ap()
    xhp_d = nc.dram_tensor("xhp", [128, NIC * F], f32, kind="ExternalInput").ap()
    xht_d = nc.dram_tensor("xht", [64, HALF], bf16, kind="ExternalInput").ap()
    wkb_d = nc.dram_tensor("wkb", [64, 128], bf16, kind="ExternalInput").ap()
    wqkb_d = nc.dram_tensor("wqkb", [128, 128], f32, kind="ExternalInput").ap()
    out_d = nc.dram_tensor("out", [128, NIC * F], f32, kind="ExternalOutput").ap()

    with (
        tc.tile_pool(name="const", bufs=1) as cp,
        tc.tile_pool(name="sb", bufs=1) as sp,
        tc.tile_pool(name="maskp", bufs=8) as mp,
    ):
        ones_row = cp.tile([1, 128], f32)
        nc.gpsimd.memset(ones_row[:], 1.0)
        ones_col16 = cp.tile([128, 1], f16)
        nc.gpsimd.memset(ones_col16[:], 1.0)
        ones_col32 = cp.tile([128, 1], f32)
        nc.gpsimd.memset(ones_col32[:], 1.0)

        # ---- input DMAs: spread descriptor generation across engines ----
        wqkb = sp.tile([128, 128], f32)
        nc.sync.dma_start(out=wqkb[:], in_=wqkb_d[:])
        wkb = sp.tile([64, 128], bf16)
        nc.scalar.dma_start(out=wkb[:], in_=wkb_d[:])
        xht = sp.tile([64, HALF], bf16)
        nc.scalar.dma_start(out=xht[:], in_=xht_d[:])
        xfp = sp.tile([128, NCH, F], f32)
        for g in range(2):  # halves, so q-building starts early
            nc.sync.dma_start(
                out=xfp[:, g * 16:(g + 1) * 16, :].rearrange("p c f -> p (c f)"),
                in_=xfp_d[:, g * 16 * F:(g + 1) * 16 * F])
        xhp = sp.tile([128, NIC, F], f32)
        nc.gpsimd.dma_start(
            out=xhp[:, :, :].rearrange("p c f -> p (c f)"), in_=xhp_d[:])

        # ---- persistent sbuf ----
        q = sp.tile([128, NCH], f32)
        negq = sp.tile([128, NCH], f32)
        kk = sp.tile([128, NIC], f32)
        eq = sp.tile([128, NCH], f32)
        eq2 = sp.tile([128, NCH], f32)
        e = sp.tile([128, NIC], f32)
        k_bcast = sp.tile([128, HALF], f16)
        W = sp.tile([128, NCH, 130], f16)   # [U|u|V|v] * exp(-qm) scaling
        S_all = sp.tile([128, NIC, 130], f32)
        C1 = sp.tile([128, NIC, 65], f32)
        C_all = sp.tile([128, NIC, 65], f32)
        rr = sp.tile([128, NIC], f32)
        o_sb = sp.tile([128, NIC, F], f32)
        gb_sb = sp.tile([128, 130], f32)
        gam = sp.tile([1, 130], f32)
        dlt = sp.tile([1, 130], f32)
        G_sb = sp.tile([1, 130], f32)
        junkq = sp.tile([128, F], f32)
        junkk = sp.tile([128, F], f32)
        qm_neg = sp.tile([128, 1], f32)
        eb2 = sp.tile([128, 1], f32)
        eb8 = sp.tile([128, 1], f32)
        qm1 = sp.tile([128, 1], f32)
        qrow = sp.tile([128, 1], f32)

        with tc.tile_pool(name="pre_ps", bufs=1, space="PSUM") as pp:
            # k broadcast to all partitions in ONE matmul stage:
            # kbp[p, i] = sum_f wkb[f, p] * xht[f, i] = k_i  (wkb cols all = wk)
            kbp = pp.tile([128, 4, 512], f32)
            for g in range(4):
                nc.tensor.matmul(kbp[:, g, :], wkb[:],
                                 xht[:, g * 512:(g + 1) * 512], start=True, stop=True)
                nc.scalar.copy(k_bcast[:, g * 512:(g + 1) * 512], kbp[:, g, :])

            # q[p,c] = sum_f x[c*128+p, f] * wq[f]  (accumulating dot per chunk)
            wqb = wqkb[:, 0:64]
            wkb2 = wqkb[:, 64:128]
            jps = pp.tile([1, 64], f32)   # PE keep-warm target
            for c in range(NCH):
                nc.vector.scalar_tensor_tensor(
                    junkq[:], xfp[:, c, :], 1.0, wqb,
                    Alu.mult, Alu.mult, accum_out=q[:, c:c + 1])
                if c % 4 == 3:
                    # tiny dummy matmul keeps the PE HAM window busy pre-stream
                    nc.tensor.matmul(jps[0:1, 0:1], ones_col32[:], q[:, c:c + 1],
                                     start=True, stop=True)
            nc.vector.tensor_scalar(negq[:], q[:], -1.0, None, Alu.mult)

            # qm = max(q); shifted exponentials (fp16-safe)
            nc.vector.tensor_reduce(qrow[:], q[:], Ax.X, Alu.max)
            nc.gpsimd.partition_all_reduce(qm1[:], qrow[:], channels=128,
                                           reduce_op=bass_isa.ReduceOp.max)
            nc.vector.tensor_scalar(qm_neg[:], qm1[:], -1.0, None, Alu.mult)
            nc.vector.tensor_scalar(eb2[:], qm_neg[:], 0.2, None, Alu.mult)
            nc.scalar.activation(eq[:], q[:], Act.Exp, bias=qm_neg[:])
            nc.scalar.activation(eq2[:], q[:], Act.Exp, bias=eb2[:], scale=0.2)

        # ---- main PSUM pool: 7 banks of accumulators + bank 8 (gamma, G) ----
        with tc.tile_pool(name="mmps", bufs=1, space="PSUM") as mps:
            mm = mps.tile([128, 7, 512], f32)
            gps = mps.tile([128, 512], f32)

            # W build (fp16) in 4 groups of 8 chunks; gamma/delta column-sum
            # matmuls follow each group so G is ready mid-stream.
            # u/v scale columns: two wide strided copies.
            nc.vector.tensor_copy(W[:, :, 64:65], eq[:].rearrange("p (c o) -> p c o", o=1))
            nc.vector.tensor_copy(W[:, :, 129:130], eq2[:].rearrange("p (c o) -> p c o", o=1))
            for g in range(4):
                cs = slice(g * 8, (g + 1) * 8)
                eq_s = eq[:, cs]
                eq2_s = eq2[:, cs]
                eq_b = dataclasses.replace(eq_s, ap=[eq_s.ap[0], eq_s.ap[1], [0, F]])
                eq2_b = dataclasses.replace(eq2_s, ap=[eq2_s.ap[0], eq2_s.ap[1], [0, F]])
                nc.vector.tensor_tensor(W[:, cs, 0:64], xfp[:, cs, :], eq_b, Alu.mult)
                nc.vector.tensor_tensor(W[:, cs, 65:129], xfp[:, cs, :], eq2_b, Alu.mult)
                for c in range(g * 8, (g + 1) * 8):
                    goff = 0 if c in ACTSET else SLOT_OFF
                    last = (c == 28) if c in ACTSET else (c == NCH - 1)
                    nc.tensor.matmul(gps[0:1, goff:goff + 130], ones_col16[:],
                                     W[:, c, :], start=(c == 0), stop=last,
                                     skip_group_check=True)

            # G correction: G_U = gam_U ; G_V = -2*dlt_V - gam_V
            nc.vector.tensor_copy(gam[:], gps[0:1, 0:130])
            nc.vector.tensor_copy(dlt[:], gps[0:1, SLOT_OFF:SLOT_OFF + 130])
            nc.vector.tensor_copy(G_sb[:, 0:65], gam[:, 0:65])
            nc.vector.scalar_tensor_tensor(
                G_sb[:, 65:130], dlt[:, 65:130], -2.0, gam[:, 65:130],
                Alu.mult, Alu.subtract)
            # broadcast G down partitions (bank-8 columns 306:436 — disjoint
            # from both gamma slots so every has_written bit there is clear)
            nc.tensor.matmul(gps[:, 306:436], ones_row[:], G_sb[:],
                             start=False, stop=True, skip_group_check=True)
            nc.vector.tensor_copy(gb_sb[:], gps[:, 306:436])

            # ---- main masked matmuls: 16 accumulators in 7 banks ----
            # i-chunk ic -> bank ic % 7, column offset (ic // 7) * SLOT_OFF.
            for c in range(NCH):
                m = mp.tile([128, HALF], f16, tag="mask")
                if c in ACTSET:
                    nc.scalar.activation(m[:], k_bcast[:], Act.Sign, bias=q[:, c:c + 1])
                elif c in GPSET:
                    nc.gpsimd.tensor_scalar(m[:], k_bcast[:],
                                            negq[:, c:c + 1], 2.0, Alu.is_gt, Alu.mult)
                else:
                    nc.vector.tensor_scalar(m[:], k_bcast[:],
                                            negq[:, c:c + 1], 2.0, Alu.is_gt, Alu.mult)
                for ic in range(NIC):
                    bank, slot = ic % 7, ic // 7
                    off = slot * SLOT_OFF
                    nc.tensor.matmul(mm[:, bank, off:off + 130],
                                     m[:, ic * 128:(ic + 1) * 128],
                                     W[:, c, :],
                                     start=(c == 0 and slot == 0),
                                     stop=(c == NCH - 1),
                                     skip_group_check=True)
                if NIC <= c < 2 * NIC:  # k dot-products fill DVE bubbles late
                    nc.vector.scalar_tensor_tensor(
                        junkk[:], xhp[:, c - NIC, :], 1.0, wkb2,
                        Alu.mult, Alu.mult, accum_out=kk[:, c - NIC:c - NIC + 1])
                if c == 2 * NIC - 1:
                    nc.vector.tensor_scalar(eb8[:], qm_neg[:], 0.8, None, Alu.mult)
                    nc.scalar.activation(e[:], kk[:], Act.Exp, bias=eb8[:], scale=-0.8)

            # ---- evacuate with the G add folded in (PSUM src is 1x anyway) ----
            gb3 = gb_sb[:].rearrange("p (o f) -> p o f", o=1)
            for lo, hi, boff in ((0, 7, 0), (7, 14, SLOT_OFF), (14, 16, 2 * SLOT_OFF)):
                nsl = hi - lo
                gb_b = dataclasses.replace(gb3, ap=[gb3.ap[0], [0, nsl], gb3.ap[2]])
                nc.vector.tensor_tensor(S_all[:, lo:hi, :],
                                        mm[:, 0:nsl, boff:boff + 130], gb_b, Alu.add)

        # ---- post: C = e*S_V - S_U ; rr = 1/C[:,64] ; out = C[:, :64]*rr ----
        e3 = e[:].rearrange("p (c o) -> p c o", o=1)
        e_b = dataclasses.replace(e3, ap=[e3.ap[0], e3.ap[1], [0, 65]])
        nc.vector.tensor_tensor(C1[:], S_all[:, :, 65:130], e_b, Alu.mult)
        nc.vector.tensor_tensor(C_all[:], C1[:], S_all[:, :, 0:65], Alu.subtract)
        nc.vector.reciprocal(rr[:], C_all[:, :, 64:65])
        rr3 = rr[:].rearrange("p (c o) -> p c o", o=1)
        rr_b = dataclasses.replace(rr3, ap=[rr3.ap[0], rr3.ap[1], [0, F]])
        nc.vector.tensor_tensor(o_sb[:], C_all[:, :, 0:64], rr_b, Alu.mult)
        nc.sync.dma_start(out=out_d[:], in_=o_sb[:])


def build_program():
    if "nc" in _CACHE:
        return _CACHE["nc"]
    from concourse import bacc, tile

    nc = bacc.Bacc("TRN2", target_bir_lowering=False, debug=False,
                   enable_asserts=True, num_devices=8)
    with tile.TileContext(nc) as tc:
        _body(nc, tc)
    nc.compile()
    _CACHE["nc"] = nc
    return nc


def make_in_maps(x, weight_key, weight_query):
    x = np.ascontiguousarray(np.asarray(x, dtype=np.float32))
    wk = np.asarray(weight_key, dtype=np.float32).reshape(-1)
    wq = np.asarray(weight_query, dtype=np.float32).reshape(-1)
    import ml_dtypes
    wkb = np.ascontiguousarray(
        np.repeat(wk[:, None], 128, axis=1).astype(ml_dtypes.bfloat16))  # [64, 128]
    wqkb = np.ascontiguousarray(
        np.tile(np.concatenate([wq, wk])[None, :], (128, 1))).astype(np.float32)
    in_maps = []
    for core in range(8):
        b, h = divmod(core, 2)
        xb = x[b]                                    # [N, F]
        xh = xb[h * HALF:(h + 1) * HALF]             # [HALF, F]
        xfp = np.ascontiguousarray(
            xb.reshape(NCH, 128, F).transpose(1, 0, 2).reshape(128, NCH * F))
        xhp = np.ascontiguousarray(
            xh.reshape(NIC, 128, F).transpose(1, 0, 2).reshape(128, NIC * F))
        in_maps.append({
            "xfp": xfp,
            "xhp": xhp,
            "xht": np.ascontiguousarray(xh.T.astype(ml_dtypes.bfloat16)),  # [64, HALF]
            "wkb": wkb,
            "wqkb": wqkb,
        })
    return in_maps


def assemble(results):
    out = np.empty((BZ, F, N), dtype=np.float32)
    for core in range(8):
        b, h = divmod(core, 2)
        o = results[core]["out"].reshape(128, NIC, F)        # [p, ic, f]
        # i_local = ic*128 + p  ->  [f, ic, p] then flatten
        out[b, :, h * HALF:(h + 1) * HALF] = o.transpose(2, 1, 0).reshape(F, HALF)
    return out


def kernel(x, weight_key, weight_query, _trace=False, _tmpdir=None):
    from concourse.bass_utils import run_bass_kernel_spmd

    nc = build_program()
    in_maps = make_in_maps(x, weight_key, weight_query)
    res = run_bass_kernel_spmd(nc, in_maps, core_ids=list(range(8)), trace=_trace,
                               tmpdir=_tmpdir)
    out = assemble(res.results)
    if _trace:
        return out, res
    return out


# revision 6
# speedup vs baseline: 5.0404x; 5.0404x over previous
"""GAT additive-attention kernel (nn_GAT) for 8 Trainium2 NeuronCores.

reference:
    k = x @ w_k; q = x @ w_q                      # [bz, N, 1]
    s[b,i,j]   = leaky_relu(k[b,i] + q[b,j], 0.2)
    attn       = softmax(s, axis=j)
    out        = (attn @ x).transpose(0, 2, 1)    # [bz, F, N]

Key identity: with sigma[i,j] = sign(k_i + q_j),
    exp(lrelu(s)) = exp(k_i)exp(q_j)       if s > 0
                  = exp(.2 k_i)exp(.2 q_j) if s <= 0
so with U = exp(q_j - qm)*[x_j|1], V = exp(.2(q_j - qm))*[x_j|1]:
    out_i = num_i / den_i,  [num|den]_i = (M@U)_i + e_i * (colsum(V) - (M@V))_i
where M = (sigma+1)/2 and e_i = exp(-.8 (k_i + qm)).  Everything reduces
to ONE N x N masked matmul  T = sigma-ish @ [U|V]  (fp16, rescaled by the
global qm = max q so fp16 never overflows) plus O(N*F) pre/post work.

v2 schedule (vs v1): the host pre-broadcasts wq/wk so the kernel needs no
weight-broadcast matmuls; k_bcast comes from ONE matmul stage (wk replicated
as the stationary operand against x^T); q is built chunk-by-chunk with
accumulating scalar_tensor_tensor as the x DMA lands; gamma/delta column
sums run right after W is built (not at stream end) so the whole G
correction is ready mid-stream; the G add is folded into the PSUM
evacuation (tensor_tensor add, PSUM src is 1x anyway); the post phase uses
wide 3D ops instead of per-chunk ops; masks are produced on three engines
(ACT / DVE / GPSIMD) so production always stays ahead of the matmul stream.

All 16 row-chunk accumulators live in PSUM simultaneously by packing
2-3 accumulation groups per bank: start=True (which clears the WHOLE
bank's has_written bits) is only used by the first group touching each
bank; later groups open with start=False, which overwrites where the
bits are clear and accumulates afterwards.

Sharding: core c handles batch b = c//2, row-half h = c%2 (2048 rows),
fully data-parallel (no collectives).
"""

import sys
import numpy as np

for _p in ("/opt/trn_rl_repo",):
    if _p not in sys.path:
        sys.path.insert(0, _p)

N = 4096
F = 64
BZ = 4
HALF = 2048
NCH = 32          # j-chunks of 128 (full N)
NIC = 16          # i-chunks of 128 (this core's half)
NEG_SLOPE = 0.2
SLOT_OFF = 176    # fp32 elems between accumulation groups within a bank

ACTSET = frozenset(c for c in range(NCH) if c % 8 < 3)    # sign masks, slot A
LAST_ACT = max(ACTSET)

_CACHE = {}


def _body(nc, tc):
    import dataclasses
    import concourse.mybir as mybir
    from concourse import bass_isa

    f32 = mybir.dt.float32
    f16 = mybir.dt.float16
    bf16 = mybir.dt.bfloat16
    Alu = mybir.AluOpType
    Act = mybir.ActivationFunctionType
    Ax = mybir.AxisListType

    # host-prepacked inputs (see make_in_maps)
    xfp_d = nc.dram_tensor("xfp", [128, NCH * F], f32, kind="ExternalInput").ap()
    xhp_d = nc.dram_tensor("xhp", [128, NIC * F], f32, kind="ExternalInput").ap()
    xht_d = nc.dram_tensor("xht", [64, HALF], bf16, kind="ExternalInput").ap()
    wkb_d = nc.dram_tensor("wkb", [64, 128], bf16, kind="ExternalInput").ap()
    wqkb_d = nc.dram_tensor("wqkb", [128, 128], f32, kind="ExternalInput").ap()
    out_d = nc.dram_tensor("out", [128, NIC * F], f32, kind="ExternalOutput").ap()

    with (
        tc.tile_pool(name="const", bufs=1) as cp,
        tc.tile_pool(name="sb", bufs=1) as sp,
        tc.tile_pool(name="maskp", bufs=8) as mp,
    ):
        ones_row = cp.tile([1, 128], f32)
        nc.gpsimd.memset(ones_row[:], 1.0)
        ones_col16 = cp.tile([128, 1], f16)
        nc.gpsimd.memset(ones_col16[:], 1.0)
        ones_col32 = cp.tile([128, 1], f32)
        nc.gpsimd.memset(ones_col32[:], 1.0)

        # ---- input DMAs: spread descriptor generation across engines ----
        wqkb = sp.tile([128, 128], f32)
        nc.sync.dma_start(out=wqkb[:], in_=wqkb_d[:])
        wkb = sp.tile([64, 128], bf16)
        nc.scalar.dma_start(out=wkb[:], in_=wkb_d[:])
        xht = sp.tile([64, HALF], bf16)
        nc.scalar.dma_start(out=xht[:], in_=xht_d[:])
        xfp = sp.tile([128, NCH, F], f32)
        for g in range(2):  # halves, so q-building starts early
            nc.sync.dma_start(
                out=xfp[:, g * 16:(g + 1) * 16, :].rearrange("p c f -> p (c f)"),
                in_=xfp_d[:, g * 16 * F:(g + 1) * 16 * F])
        xhp = sp.tile([128, NIC, F], f32)
        nc.gpsimd.dma_start(
            out=xhp[:, :, :].rearrange("p c f -> p (c f)"), in_=xhp_d[:])

        # ---- persistent sbuf ----
        q = sp.tile([128, NCH], f32)
        negq = sp.tile([128, NCH], f32)
        kk = sp.tile([128, NIC], f32)
        eq = sp.tile([128, NCH], f32)
        eq2 = sp.tile([128, NCH], f32)
        e = sp.tile([128, NIC], f32)
        k_bcast = sp.tile([128, HALF], f16)
        W = sp.tile([128, NCH, 130], f16)   # [U|u|V|v] * exp(-qm) scaling
        S_all = sp.tile([128, NIC, 130], f32)
        C1 = sp.tile([128, NIC, 65], f32)
        C_all = sp.tile([128, NIC, 65], f32)
        rr = sp.tile([128, NIC], f32)
        o_sb = sp.tile([128, NIC, F], f32)
        gb_sb = sp.tile([128, 130], f32)
        gam = sp.tile([1, 130], f32)
        dlt = sp.tile([1, 130], f32)
        G_sb = sp.tile([1, 130], f32)
        junkq = sp.tile([128, F], f32)
        junkk = sp.tile([128, F], f32)
        qm_neg = sp.tile([128, 1], f32)
        eb2 = sp.tile([128, 1], f32)
        eb8 = sp.tile([128, 1], f32)
        qm1 = sp.tile([128, 1], f32)
        qrow = sp.tile([128, 1], f32)

        with tc.tile_pool(name="pre_ps", bufs=1, space="PSUM") as pp:
            # k broadcast to all partitions in ONE matmul stage:
            # kbp[p, i] = sum_f wkb[f, p] * xht[f, i] = k_i  (wkb cols all = wk)
            kbp = pp.tile([128, 4, 512], f32)
            for g in range(4):
                nc.tensor.matmul(kbp[:, g, :], wkb[:],
                                 xht[:, g * 512:(g + 1) * 512], start=True, stop=True)
                nc.scalar.copy(k_bcast[:, g * 512:(g + 1) * 512], kbp[:, g, :])

            # q[p,c] = sum_f x[c*128+p, f] * wq[f]  (accumulating dot per chunk)
            wqb = wqkb[:, 0:64]
            wkb2 = wqkb[:, 64:128]
            jps = pp.tile([1, 64], f32)   # PE keep-warm target
            for c in range(NCH):
                nc.vector.scalar_tensor_tensor(
                    junkq[:], xfp[:, c, :], 1.0, wqb,
                    Alu.mult, Alu.mult, accum_out=q[:, c:c + 1])
                if c % 4 == 3:
                    # tiny dummy matmul keeps the PE HAM window busy pre-stream
                    nc.tensor.matmul(jps[0:1, 0:1], ones_col32[:], q[:, c:c + 1],
                                     start=True, stop=True)
            nc.vector.tensor_scalar(negq[:], q[:], -1.0, None, Alu.mult)

            # qm = max(q); shifted exponentials (fp16-safe)
            nc.vector.tensor_reduce(qrow[:], q[:], Ax.X, Alu.max)
            nc.gpsimd.partition_all_reduce(qm1[:], qrow[:], channels=128,
                                           reduce_op=bass_isa.ReduceOp.max)
            nc.vector.tensor_scalar(qm_neg[:], qm1[:], -1.0, None, Alu.mult)
            nc.vector.tensor_scalar(eb2[:], qm_neg[:], 0.2, None, Alu.mult)
            nc.scalar.activation(eq[:], q[:], Act.Exp, bias=qm_neg[:])
            nc.scalar.activation(eq2[:], q[:], Act.Exp, bias=eb2[:], scale=0.2)

        # ---- main PSUM pool: 7 banks of accumulators + bank 8 (gamma, G) ----
        with tc.tile_pool(name="mmps", bufs=1, space="PSUM") as mps:
            mm = mps.tile([128, 7, 512], f32)
            gps = mps.tile([128, 512], f32)

            # W build (fp16) in 4 groups of 8 chunks; gamma/delta column-sum
            # matmuls follow each group so G is ready mid-stream.
            # u/v scale columns: two wide strided copies.
            nc.vector.tensor_copy(W[:, :, 64:65], eq[:].rearrange("p (c o) -> p c o", o=1))
            nc.vector.tensor_copy(W[:, :, 129:130], eq2[:].rearrange("p (c o) -> p c o", o=1))
            for g in range(4):
                cs = slice(g * 8, (g + 1) * 8)
                eq_s = eq[:, cs]
                eq2_s = eq2[:, cs]
                eq_b = dataclasses.replace(eq_s, ap=[eq_s.ap[0], eq_s.ap[1], [0, F]])
                eq2_b = dataclasses.replace(eq2_s, ap=[eq2_s.ap[0], eq2_s.ap[1], [0, F]])
                nc.vector.tensor_tensor(W[:, cs, 0:64], xfp[:, cs, :], eq_b, Alu.mult)
                nc.vector.tensor_tensor(W[:, cs, 65:129], xfp[:, cs, :], eq2_b, Alu.mult)
                for c in range(g * 8, (g + 1) * 8):
                    goff = 0 if c in ACTSET else SLOT_OFF
                    last = (c == LAST_ACT) if c in ACTSET else (c == NCH - 1)
                    nc.tensor.matmul(gps[0:1, goff:goff + 130], ones_col16[:],
                                     W[:, c, :], start=(c == 0), stop=last,
                                     skip_group_check=True)

            # G correction: G_U = gam_U ; G_V = -2*dlt_V - gam_V
            nc.vector.tensor_copy(gam[:], gps[0:1, 0:130])
            nc.vector.tensor_copy(dlt[:], gps[0:1, SLOT_OFF:SLOT_OFF + 130])
            nc.vector.tensor_copy(G_sb[:, 0:65], gam[:, 0:65])
            nc.vector.scalar_tensor_tensor(
                G_sb[:, 65:130], dlt[:, 65:130], -2.0, gam[:, 65:130],
                Alu.mult, Alu.subtract)
            # broadcast G down partitions (bank-8 columns 306:436 — disjoint
            # from both gamma slots so every has_written bit there is clear)
            nc.tensor.matmul(gps[:, 306:436], ones_row[:], G_sb[:],
                             start=False, stop=True, skip_group_check=True)
            nc.vector.tensor_copy(gb_sb[:], gps[:, 306:436])

            # ---- main masked matmuls: 16 accumulators in 7 banks ----
            # i-chunk ic -> bank ic % 7, column offset (ic // 7) * SLOT_OFF.
            for c in range(NCH):
                m = mp.tile([128, HALF], f16, tag="mask")
                if c in ACTSET:
                    nc.scalar.activation(m[:], k_bcast[:], Act.Sign, bias=q[:, c:c + 1])
                else:
                    nc.vector.tensor_scalar(m[:], k_bcast[:],
                                            negq[:, c:c + 1], 2.0, Alu.is_gt, Alu.mult)
                for ic in range(NIC):
                    bank, slot = ic % 7, ic // 7
                    off = slot * SLOT_OFF
                    nc.tensor.matmul(mm[:, bank, off:off + 130],
                                     m[:, ic * 128:(ic + 1) * 128],
                                     W[:, c, :],
                                     start=(c == 0 and slot == 0),
                                     stop=(c == NCH - 1),
                                     skip_group_check=True)
                if NIC <= c < 2 * NIC:  # k dot-products fill DVE bubbles late
                    nc.vector.scalar_tensor_tensor(
                        junkk[:], xhp[:, c - NIC, :], 1.0, wkb2,
                        Alu.mult, Alu.mult, accum_out=kk[:, c - NIC:c - NIC + 1])
                if c == 2 * NIC - 1:
                    nc.vector.tensor_scalar(eb8[:], qm_neg[:], 0.8, None, Alu.mult)
                    nc.scalar.activation(e[:], kk[:], Act.Exp, bias=eb8[:], scale=-0.8)

            # ---- evacuate with the G add folded in (PSUM src is 1x anyway) ----
            gb3 = gb_sb[:].rearrange("p (o f) -> p o f", o=1)
            for lo, hi, boff in ((0, 7, 0), (7, 14, SLOT_OFF), (14, 16, 2 * SLOT_OFF)):
                nsl = hi - lo
                gb_b = dataclasses.replace(gb3, ap=[gb3.ap[0], [0, nsl], gb3.ap[2]])
                nc.vector.tensor_tensor(S_all[:, lo:hi, :],
                                        mm[:, 0:nsl, boff:boff + 130], gb_b, Alu.add)

        # ---- post: C = e*S_V - S_U ; rr = 1/C[:,64] ; out = C[:, :64]*rr ----
        e3 = e[:].rearrange("p (c o) -> p c o", o=1)
        e_b = dataclasses.replace(e3, ap=[e3.ap[0], e3.ap[1], [0, 65]])
        nc.vector.tensor_tensor(C1[:], S_all[:, :, 65:130], e_b, Alu.mult)
        nc.vector.tensor_tensor(C_all[:], C1[:], S_all[:, :, 0:65], Alu.subtract)
        nc.vector.reciprocal(rr[:], C_all[:, :, 64:65])
        rr3 = rr[:].rearrange("p (c o) -> p c o", o=1)
        rr_b = dataclasses.replace(rr3, ap=[rr3.ap[0], rr3.ap[1], [0, F]])
        nc.vector.tensor_tensor(o_sb[:], C_all[:, :, 0:64], rr_b, Alu.mult)
        nc.sync.dma_start(out=out_d[:], in_=o_sb[:])


def build_program():
    if "nc" in _CACHE:
        return _CACHE["nc"]
    from concourse import bacc, tile

    nc = bacc.Bacc("TRN2", target_bir_lowering=False, debug=False,
                   enable_asserts=True, num_devices=8)
    with tile.TileContext(nc) as tc:
        _body(nc, tc)
    nc.compile()
    _CACHE["nc"] = nc
    return nc


def make_in_maps(x, weight_key, weight_query):
    x = np.ascontiguousarray(np.asarray(x, dtype=np.float32))
    wk = np.asarray(weight_key, dtype=np.float32).reshape(-1)
    wq = np.asarray(weight_query, dtype=np.float32).reshape(-1)
    import ml_dtypes
    wkb = np.ascontiguousarray(
        np.repeat(wk[:, None], 128, axis=1).astype(ml_dtypes.bfloat16))  # [64, 128]
    wqkb = np.ascontiguousarray(
        np.tile(np.concatenate([wq, wk])[None, :], (128, 1))).astype(np.float32)
    in_maps = []
    for core in range(8):
        b, h = divmod(core, 2)
        xb = x[b]                                    # [N, F]
        xh = xb[h * HALF:(h + 1) * HALF]             # [HALF, F]
        xfp = np.ascontiguousarray(
            xb.reshape(NCH, 128, F).transpose(1, 0, 2).reshape(128, NCH * F))
        xhp = np.ascontiguousarray(
            xh.reshape(NIC, 128, F).transpose(1, 0, 2).reshape(128, NIC * F))
        in_maps.append({
            "xfp": xfp,
            "xhp": xhp,
            "xht": np.ascontiguousarray(xh.T.astype(ml_dtypes.bfloat16)),  # [64, HALF]
            "wkb": wkb,
            "wqkb": wqkb,
        })
    return in_maps


def assemble(results):
    out = np.empty((BZ, F, N), dtype=np.float32)
    for core in range(8):
        b, h = divmod(core, 2)
        o = results[core]["out"].reshape(128, NIC, F)        # [p, ic, f]
        # i_local = ic*128 + p  ->  [f, ic, p] then flatten
        out[b, :, h * HALF:(h + 1) * HALF] = o.transpose(2, 1, 0).reshape(F, HALF)
    return out


def kernel(x, weight_key, weight_query, _trace=False, _tmpdir=None):
    from concourse.bass_utils import run_bass_kernel_spmd

    nc = build_program()
    in_maps = make_in_maps(x, weight_key, weight_query)
    res = run_bass_kernel_spmd(nc, in_maps, core_ids=list(range(8)), trace=_trace,
                               tmpdir=_tmpdir)
    out = assemble(res.results)
    if _trace:
        return out, res
    return out
